# revision 1
# baseline (speedup 1.0000x reference)
"""Trainium2 Bass kernel for nn_L3_31799937859925 (sparse_attention).

Strategy:
- Each query row (label = seq_sort[j] in [0,64)) attends only to kv rows with
  emb_alloc == label, so we sort queries by label on the host and give each of
  the 8 cores a contiguous 2048-query slice (pure data parallel, no
  collectives). kv rows are label-sorted too, so each 512-query tile only needs
  a small contiguous kv window (W columns) + an additive -1e30 mask bias.
- On device everything is feature-major ([feature, query]) so no transposes are
  needed: scoresT = K'T @ x, softmax sums / rms stats via ones-column matmuls
  on the PE, per-query scalars broadcast across partitions via K=1 matmuls.
- norm_in_weight is folded into w_k, norm_out_weight into w_mix (host side).
- All heavy matmuls run in float32r (relaxed fp32, full PE rate, ~1.5e-4 rel).
"""
import numpy as np

import concourse.bass as bass
import concourse.tile as tile
from concourse import bacc, mybir
import concourse.bass_utils as bass_utils

F32 = mybir.dt.float32
F32R = mybir.dt.float32r
AF = mybir.ActivationFunctionType
MUL = mybir.AluOpType.mult
ADD = mybir.AluOpType.add

H, N_EMB, D_EMB, D_UP = 1024, 8192, 512, 2048
B, T = 4, 4096
BT = B * T                  # 16384
NC = 8                      # cores
NQ = BT // NC               # 2048 queries per core
QT = 512                    # queries per q-tile
NQT = NQ // QT              # 4 q-tiles per core
HC = H // 128               # 8
DC = D_EMB // 128           # 4
JC = D_UP // 128            # 16
KC = (D_UP + H) // 128      # 24 contraction chunks for mix
MC = H // 128               # 8 output chunks

LAST_RESULTS = None         # BassKernelResults of the most recent run (for test.py)
LAST_EXEC_S = None
_PROGRAM_CACHE = {}


def _build_program(W):
    """Build the SPMD single-core program. W = kv window width (mult of 128)."""
    n_kvc = W // 128
    nc = bacc.Bacc("TRN2", target_bir_lowering=False, debug=False,
                   enable_asserts=False)

    x_in = nc.dram_tensor("x_in", [128, HC, NQ], F32R, kind="ExternalInput")
    kt_in = nc.dram_tensor("kt_in", [NQT, 128, HC, W], F32R, kind="ExternalInput")
    v_in = nc.dram_tensor("v_in", [NQT, 128, n_kvc, D_EMB], F32R, kind="ExternalInput")
    b_in = nc.dram_tensor("b_in", [NQT, 128, n_kvc, QT], F32, kind="ExternalInput")
    wup_in = nc.dram_tensor("wup_in", [128, DC, D_UP], F32R, kind="ExternalInput")
    wmix_in = nc.dram_tensor("wmix_in", [MC, 128, KC, 128], F32R, kind="ExternalInput")
    out_d = nc.dram_tensor("out_d", [MC, 128, NQ], F32, kind="ExternalOutput")

    from contextlib import ExitStack
    with tile.TileContext(nc) as tc, ExitStack() as ctx:
        ec = ctx.enter_context
        cst = ec(tc.tile_pool(name="cst", bufs=1))
        pwup = ec(tc.tile_pool(name="wup", bufs=1))
        px = ec(tc.tile_pool(name="px", bufs=2))
        pkt = ec(tc.tile_pool(name="pkt", bufs=1))
        pv = ec(tc.tile_pool(name="pv", bufs=1))
        pb = ec(tc.tile_pool(name="pb", bufs=1))
        pwm = ec(tc.tile_pool(name="pwm", bufs=3))
        px2 = ec(tc.tile_pool(name="px2", bufs=2))
        ppu = ec(tc.tile_pool(name="ppu", bufs=1))
        pt = ec(tc.tile_pool(name="pt", bufs=3))
        pcomb = ec(tc.tile_pool(name="pcomb", bufs=1))
        pup = ec(tc.tile_pool(name="pup", bufs=1))
        pu2 = ec(tc.tile_pool(name="pu2", bufs=4))
        pbc = ec(tc.tile_pool(name="pbc", bufs=4))
        prows = ec(tc.tile_pool(name="prows", bufs=3))
        po = ec(tc.tile_pool(name="po", bufs=2))
        pbig = ec(tc.tile_pool(name="pbig", bufs=6, space="PSUM"))
        prow = ec(tc.tile_pool(name="prow", bufs=2, space="PSUM"))

        if True:
            ones_f = cst.tile([128, 1], F32)
            nc.vector.memset(ones_f, 1.0)
            ones_col = cst.tile([128, 1], F32R)
            nc.vector.tensor_copy(ones_col, ones_f)
            ones_rf = cst.tile([1, 128], F32)
            nc.vector.memset(ones_rf, 1.0)
            ones_row = cst.tile([1, 128], F32R)
            nc.vector.tensor_copy(ones_row, ones_rf)
            eps_t = cst.tile([128, 1], F32)
            nc.vector.memset(eps_t, 1e-6)

            wup_sb = pwup.tile([128, DC, D_UP], F32R)
            nc.sync.dma_start(wup_sb[:], wup_in.ap())

            for qt in range(NQT):
                qs = slice(qt * QT, (qt + 1) * QT)
                x_t = px.tile([128, HC, QT], F32R, tag="x")
                nc.sync.dma_start(x_t[:], x_in.ap()[:, :, qs])
                kt_t = pkt.tile([128, HC, W], F32R, tag="kt")
                nc.sync.dma_start(kt_t[:], kt_in.ap()[qt])
                v_t = pv.tile([128, n_kvc, D_EMB], F32R, tag="v")
                nc.sync.dma_start(v_t[:], v_in.ap()[qt])
                b_t = pb.tile([128, n_kvc, QT], F32, tag="b")
                nc.sync.dma_start(b_t[:], b_in.ap()[qt])

                # ---- rms_in stats: inv_rms per query as broadcast [128, QT]
                ss_ps = prow.tile([1, QT], F32, tag="row")
                for hc in range(HC):
                    x2 = px2.tile([128, QT], F32R, tag="x2")
                    nc.scalar.activation(x2, x_t[:, hc, :].bitcast(F32), AF.Square)
                    nc.tensor.matmul(ss_ps, lhsT=ones_col, rhs=x2,
                                     start=(hc == 0), stop=(hc == HC - 1))
                sd = prows.tile([1, QT], F32, tag="rows")
                nc.scalar.activation(sd, ss_ps, AF.Sqrt, bias=eps_t[:1],
                                     scale=1.0 / H)
                crf = prows.tile([1, QT], F32, tag="rows")
                nc.vector.reciprocal(crf, sd)
                cr = prows.tile([1, QT], F32R, tag="rowsr")
                nc.vector.tensor_copy(cr, crf)
                c_b = pbc.tile([128, QT], F32, tag="bc")

                # ---- scoresT [kv, q] per kv chunk; t = s*c + bias; pu = exp(t)
                pu_t = ppu.tile([128, n_kvc, QT], F32R, tag="pu")
                for kvc in range(n_kvc):
                    s_ps = pbig.tile([128, QT], F32, tag="big")
                    for hc in range(HC):
                        nc.tensor.matmul(
                            s_ps, lhsT=kt_t[:, hc, kvc * 128:(kvc + 1) * 128],
                            rhs=x_t[:, hc, :],
                            start=(hc == 0), stop=(hc == HC - 1))
                    if kvc == 0:
                        # emit bcast here so PE doesn't stall on the recip chain
                        cb_ps = pbig.tile([128, QT], F32, tag="big")
                        nc.tensor.matmul(cb_ps, lhsT=ones_row, rhs=cr,
                                         start=True, stop=True)
                        nc.vector.tensor_copy(c_b, cb_ps)
                    t_sb = pt.tile([128, QT], F32, tag="t")
                    nc.vector.tensor_tensor(t_sb, s_ps, c_b, MUL)
                    nc.vector.tensor_tensor(t_sb, t_sb, b_t[:, kvc, :], ADD)
                    nc.scalar.activation(pu_t[:, kvc, :], t_sb, AF.Exp)

                # ---- z = sum_kv pu ; z_b = 1/z broadcast
                z_ps = prow.tile([1, QT], F32, tag="row")
                for kvc in range(n_kvc):
                    nc.tensor.matmul(z_ps, lhsT=ones_col, rhs=pu_t[:, kvc, :],
                                     start=(kvc == 0), stop=(kvc == n_kvc - 1))
                zrf = prows.tile([1, QT], F32, tag="rows")
                nc.vector.reciprocal(zrf, z_ps)
                zr = prows.tile([1, QT], F32R, tag="rowsr")
                nc.vector.tensor_copy(zr, zrf)
                z_b = pbc.tile([128, QT], F32, tag="bc")

                # ---- combT [d, q] = V^T pu, normalized by z
                comb_t = pcomb.tile([128, DC, QT], F32R, tag="comb")
                for dc in range(DC):
                    c_ps = pbig.tile([128, QT], F32, tag="big")
                    for kvc in range(n_kvc):
                        nc.tensor.matmul(
                            c_ps, lhsT=v_t[:, kvc, dc * 128:(dc + 1) * 128],
                            rhs=pu_t[:, kvc, :],
                            start=(kvc == 0), stop=(kvc == n_kvc - 1))
                    if dc == 0:
                        zb_ps = pbig.tile([128, QT], F32, tag="big")
                        nc.tensor.matmul(zb_ps, lhsT=ones_row, rhs=zr,
                                         start=True, stop=True)
                        nc.vector.tensor_copy(z_b, zb_ps)
                    nc.vector.tensor_tensor(comb_t[:, dc, :], c_ps, z_b, MUL)

                # ---- upT [j, q] (raw, pre-norm) + sum of squares
                up_t = pup.tile([128, JC, QT], F32R, tag="up")
                ssu_ps = prow.tile([1, QT], F32, tag="row")
                pend = None
                for m in range(JC):
                    u_ps = pbig.tile([128, QT], F32, tag="big")
                    for dc in range(DC):
                        nc.tensor.matmul(
                            u_ps, lhsT=wup_sb[:, dc, m * 128:(m + 1) * 128],
                            rhs=comb_t[:, dc, :],
                            start=(dc == 0), stop=(dc == DC - 1))
                    if pend is not None:
                        nc.tensor.matmul(ssu_ps, lhsT=ones_col, rhs=pend,
                                         start=(m == 1), stop=False)
                    nc.vector.tensor_copy(up_t[:, m, :], u_ps)
                    u2 = pu2.tile([128, QT], F32R, tag="u2")
                    nc.scalar.activation(u2, u_ps, AF.Square)
                    pend = u2
                nc.tensor.matmul(ssu_ps, lhsT=ones_col, rhs=pend,
                                 start=False, stop=True)
                sdu = prows.tile([1, QT], F32, tag="rows")
                nc.scalar.activation(sdu, ssu_ps, AF.Sqrt, bias=eps_t[:1],
                                     scale=1.0 / D_UP)
                r2f = prows.tile([1, QT], F32, tag="rows")
                nc.vector.reciprocal(r2f, sdu)
                r2 = prows.tile([1, QT], F32R, tag="rowsr")
                nc.vector.tensor_copy(r2, r2f)
                i2_b = pbc.tile([128, QT], F32, tag="bc")

                # ---- mix: out[mc] = i2_b * (Wmix_up @ up) + (Wmix_x @ x)
                for mc in range(MC):
                    wm_t = pwm.tile([128, KC, 128], F32R, tag="wm")
                    nc.sync.dma_start(wm_t[:], wmix_in.ap()[mc])
                    a_ps = pbig.tile([128, QT], F32, tag="big")
                    for kc in range(JC):
                        nc.tensor.matmul(a_ps, lhsT=wm_t[:, kc, :],
                                         rhs=up_t[:, kc, :],
                                         start=(kc == 0), stop=(kc == JC - 1))
                    b_ps = pbig.tile([128, QT], F32, tag="big")
                    for kc in range(MC):
                        nc.tensor.matmul(b_ps, lhsT=wm_t[:, JC + kc, :],
                                         rhs=x_t[:, kc, :],
                                         start=(kc == 0), stop=(kc == MC - 1))
                    if mc == 0:
                        i2_ps = pbig.tile([128, QT], F32, tag="big")
                        nc.tensor.matmul(i2_ps, lhsT=ones_row, rhs=r2,
                                         start=True, stop=True)
                        nc.vector.tensor_copy(i2_b, i2_ps)
                    o_sb = po.tile([128, QT], F32, tag="o")
                    nc.vector.tensor_tensor(o_sb, a_ps, i2_b, MUL)
                    nc.vector.tensor_tensor(o_sb, o_sb, b_ps, ADD)
                    nc.sync.dma_start(out_d.ap()[mc][:, qs], o_sb[:])

    nc.compile()
    return nc


def _get_program(W):
    if W not in _PROGRAM_CACHE:
        _PROGRAM_CACHE[W] = _build_program(W)
    return _PROGRAM_CACHE[W]


def kernel(**inputs) -> np.ndarray:
    global LAST_RESULTS
    inp = np.asarray(inputs["input"], np.float32)
    fw = np.asarray(inputs["fw"]).astype(np.int64)
    seq_sort = np.asarray(inputs["seq_sort"]).astype(np.int64)
    keep_cols = np.asarray(inputs["keep_cols"]).astype(np.int64)
    emb_alloc = np.asarray(inputs["emb_alloc"]).astype(np.int64)
    starts = np.asarray(inputs["starts"]).astype(np.int64)
    ends = np.asarray(inputs["ends"]).astype(np.int64)
    bb = int(np.asarray(inputs["bb"]))
    w_k = np.asarray(inputs["w_k_weight"], np.float32)
    w_v = np.asarray(inputs["w_v_weight"], np.float32)
    w_up = np.asarray(inputs["w_up_weight"], np.float32)
    w_mix = np.asarray(inputs["w_mix_weight"], np.float32)
    w_in = np.asarray(inputs["norm_in_weight"], np.float32)
    w_out = np.asarray(inputs["norm_out_weight"], np.float32)

    x = inp.reshape(BT, H)
    nb = BT // bb
    st = starts.reshape(nb, bb).min(axis=1)
    en = ends.reshape(nb, bb).max(axis=1)

    # sort block-rows j by label (stable); row s of sorted space = block-row
    # order[s] = query fw[order[s]]
    order = np.argsort(seq_sort, kind="stable")
    perm = fw[order]                         # original flat query per sorted row
    lab_q = seq_sort[order]                  # label per sorted row
    blk_q = order // bb
    st_q = st[blk_q]
    en_q = en[blk_q]
    x_sorted = x[perm]                       # [BT, H]

    # kv side: keep + label-sort; fold norm_in into K
    la = emb_alloc[keep_cols]                # [M]
    M = la.shape[0]
    kv_order = np.argsort(la, kind="stable")
    la_s = la[kv_order]
    kvpos = kv_order                         # kept-position of sorted kv row
    Bm = (w_k[keep_cols] * w_in[None, :])[kv_order]   # [M, H]
    Cm = w_v[keep_cols][kv_order]            # [M, D_EMB]

    counts = np.bincount(la_s, minlength=64)
    gstart = np.concatenate([[0], np.cumsum(counts)])  # [65]

    # per-tile windows over sorted kv
    NT = BT // QT                            # 32 global q-tiles
    win = np.empty(NT, np.int64)
    need = 0
    for g in range(NT):
        l0 = lab_q[g * QT]
        l1 = lab_q[(g + 1) * QT - 1]
        win[g] = gstart[l0]
        need = max(need, gstart[l1 + 1] - gstart[l0])
    W = max(256, int(-(-need // 128) * 128))

    # padded kv arrays so windows never go OOB
    Mp = M + W
    Bm_p = np.zeros((Mp, H), np.float32); Bm_p[:M] = Bm
    Cm_p = np.zeros((Mp, D_EMB), np.float32); Cm_p[:M] = Cm
    la_p = np.full(Mp, -1, np.int64); la_p[:M] = la_s
    kvpos_p = np.full(Mp, -1, np.int64); kvpos_p[:M] = kvpos

    # mask bias per (sorted row, window col)
    kvi = win[:, None] + np.arange(W)[None, :]           # [NT, W]
    la_w = la_p[kvi]                                     # [NT, W]
    kp_w = kvpos_p[kvi]
    lab_t = lab_q.reshape(NT, QT)
    st_t = st_q.reshape(NT, QT)
    en_t = en_q.reshape(NT, QT)
    valid = ((la_w[:, None, :] == lab_t[:, :, None])
             & (kp_w[:, None, :] >= st_t[:, :, None])
             & (kp_w[:, None, :] < en_t[:, :, None]))    # [NT, QT, W]
    bias = np.where(valid, np.float32(0), np.float32(-1e30))

    KT_full = np.ascontiguousarray(Bm_p.T)               # [H, Mp]

    wm = w_mix.copy()
    wm[:, :D_UP] *= w_out[None, :]
    WmixT = np.ascontiguousarray(wm.T)                   # [3072, H]
    wmix_host = np.ascontiguousarray(
        WmixT.reshape(KC, 128, MC, 128).transpose(2, 1, 0, 3))  # [MC,128,KC,128]
    WupT = np.ascontiguousarray(w_up.T)                  # [D_EMB, D_UP]
    wup_host = np.ascontiguousarray(
        WupT.reshape(DC, 128, D_UP).transpose(1, 0, 2))  # [128, DC, D_UP]

    n_kvc = W // 128
    in_maps = []
    for c in range(NC):
        rows = slice(c * NQ, (c + 1) * NQ)
        x_c = np.ascontiguousarray(
            x_sorted[rows].T.reshape(HC, 128, NQ).transpose(1, 0, 2))  # [128,HC,NQ]
        kt_c = np.empty((NQT, 128, HC, W), np.float32)
        v_c = np.empty((NQT, 128, n_kvc, D_EMB), np.float32)
        b_c = np.empty((NQT, 128, n_kvc, QT), np.float32)
        for qt in range(NQT):
            g = c * NQT + qt
            w0 = win[g]
            kt_c[qt] = KT_full[:, w0:w0 + W].reshape(HC, 128, W).transpose(1, 0, 2)
            v_c[qt] = Cm_p[w0:w0 + W].reshape(n_kvc, 128, D_EMB).transpose(1, 0, 2)
            b_c[qt] = bias[g].T.reshape(n_kvc, 128, QT).transpose(1, 0, 2)
        in_maps.append({
            "x_in": x_c, "kt_in": kt_c, "v_in": v_c, "b_in": b_c,
            "wup_in": wup_host, "wmix_in": wmix_host,
        })

    nc = _get_program(W)
    import time as _time
    global LAST_EXEC_S
    _t0 = _time.time()
    LAST_RESULTS = bass_utils.run_bass_kernel_spmd(nc, in_maps,
                                                   core_ids=list(range(NC)))
    LAST_EXEC_S = _time.time() - _t0
    out_sorted = np.concatenate(
        [r["out_d"].transpose(2, 0, 1).reshape(NQ, H) for r in LAST_RESULTS.results],
        axis=0)                                          # [BT, H]
    final = np.empty((BT, H), np.float32)
    final[perm] = out_sorted
    return final.reshape(B, T, H)



# revision 9
# speedup vs baseline: 2.6626x; 2.6626x over previous
"""Trainium2 Bass kernel for nn_L3_31799937859925 (sparse_attention).

Strategy (v2 — low-rank collapse of the up/mix chain):
- Each query row (label = seq_sort[j] in [0,64)) attends only to kv rows with
  emb_alloc == label, so we sort queries by label on the host and give each of
  the 8 cores a contiguous 2048-query slice (pure data parallel, no
  collectives). kv rows are label-sorted too, so each 512-query tile only needs
  a small contiguous kv window (W columns) + a multiplicative {0,1} mask.
- Because comb = softmax(scores) @ C lives in the span of each label's 64 C
  rows, the entire comb -> up -> rms_out -> mix_up chain collapses through
  host-precomputed per-window operators:
      CUW = (C @ w_up.T) @ (w_mix[:, :d_up] * w_out).T      [kv, H]
      G   = (C @ w_up.T) @ (C @ w_up.T).T / d_up            [kv, kv]
  Then with pu = masked exp(scores * inv_rms_in), z = sum_kv pu,
  w' = pu G pu^T (quadratic form), alpha = rsqrt(w' + eps * z^2):
      out = (pu * alpha) @ CUW + x @ w_mix[:, d_up:].T
  which removes ~13 of 18.8 GF/core from the device.
- Everything is feature-major ([feature, query]); per-query scalars broadcast
  across partitions via K=1 matmuls; partition reductions via ones-matmuls.
- All heavy matmuls run in bf16 (full PE rate, half DMA); accumulation f32.
- Software pipeline: the attention chain of tile t+1 is emitted before the
  output stage of tile t, so the PE never waits on the softmax scalar chain.
"""
import numpy as np
import ml_dtypes

import concourse.bass as bass
import concourse.tile as tile
from concourse import bacc, mybir
import concourse.bass_utils as bass_utils

F32 = mybir.dt.float32
F32R = mybir.dt.float32r
BF16 = mybir.dt.bfloat16
AF = mybir.ActivationFunctionType
MUL = mybir.AluOpType.mult
ADD = mybir.AluOpType.add
NP_BF16 = ml_dtypes.bfloat16

H, N_EMB, D_EMB, D_UP = 1024, 8192, 512, 2048
B, T = 4, 4096
BT = B * T                  # 16384
NC = 8                      # cores
NQ = BT // NC               # 2048 queries per core
QT = 512                    # queries per q-tile
NQT = NQ // QT              # 4 q-tiles per core
HC = H // 128               # 8
MC = H // 128               # 8 output chunks
EPS = 1e-6

LAST_RESULTS = None         # BassKernelResults of the most recent run (for test.py)
LAST_EXEC_S = None
_PROGRAM_CACHE = {}


def _build_program(W):
    """Build the SPMD single-core program. W = kv window width (mult of 128)."""
    n_kvc = W // 128
    nc = bacc.Bacc("TRN2", target_bir_lowering=False, debug=False,
                   enable_asserts=False)

    x_in = nc.dram_tensor("x_in", [128, HC, NQ], BF16, kind="ExternalInput")
    kt_in = nc.dram_tensor("kt_in", [NQT, 128, HC, W], BF16, kind="ExternalInput")
    cuw_in = nc.dram_tensor("cuw_in", [NQT, 128, n_kvc, H], BF16, kind="ExternalInput")
    g_in = nc.dram_tensor("g_in", [NQT, 128, n_kvc, W], BF16, kind="ExternalInput")
    m_in = nc.dram_tensor("m_in", [NQT, 128, n_kvc, QT], BF16, kind="ExternalInput")
    wm2_in = nc.dram_tensor("wm2_in", [128, HC, H], BF16, kind="ExternalInput")
    out_d = nc.dram_tensor("out_d", [MC, 128, NQ], BF16, kind="ExternalOutput")

    from contextlib import ExitStack
    with tile.TileContext(nc) as tc, ExitStack() as ctx:
        ec = ctx.enter_context
        cst = ec(tc.tile_pool(name="cst", bufs=1))
        pwm2 = ec(tc.tile_pool(name="wm2", bufs=1))
        px = ec(tc.tile_pool(name="px", bufs=3))
        pkt = ec(tc.tile_pool(name="pkt", bufs=2))
        pcuw = ec(tc.tile_pool(name="pcuw", bufs=3))
        pg = ec(tc.tile_pool(name="pg", bufs=2))
        pm = ec(tc.tile_pool(name="pm", bufs=2))
        px2 = ec(tc.tile_pool(name="px2", bufs=2))
        pt = ec(tc.tile_pool(name="pt", bufs=2))
        ppu = ec(tc.tile_pool(name="ppu", bufs=2))
        ppm = ec(tc.tile_pool(name="ppm", bufs=2))
        ppq = ec(tc.tile_pool(name="ppq", bufs=2))
        pptil = ec(tc.tile_pool(name="pptil", bufs=3))
        pbc = ec(tc.tile_pool(name="pbc", bufs=2))
        prows = ec(tc.tile_pool(name="prows", bufs=4))
        po = ec(tc.tile_pool(name="po", bufs=3))
        pbig = ec(tc.tile_pool(name="pbig", bufs=4, space="PSUM"))
        pout = ec(tc.tile_pool(name="pout", bufs=2, space="PSUM"))
        prow = ec(tc.tile_pool(name="prow", bufs=2, space="PSUM"))

        ones_f = cst.tile([128, 1], F32)
        nc.vector.memset(ones_f, 1.0)
        ones_bf = cst.tile([128, 1], BF16)
        nc.vector.tensor_copy(ones_bf, ones_f)
        ones_rf = cst.tile([1, 128], F32)
        nc.vector.memset(ones_rf, 1.0)
        ones_row = cst.tile([1, 128], BF16)
        nc.vector.tensor_copy(ones_row, ones_rf)
        eps_t = cst.tile([1, 1], F32)
        nc.vector.memset(eps_t, EPS)

        wm2_sb = pwm2.tile([128, HC, H], BF16)
        nc.sync.dma_start(wm2_sb[:], wm2_in.ap())

        # per-tile state carried from the attention stage to the out stage
        st_x = [None] * NQT
        st_cuw = [None] * NQT
        st_ptil = [None] * NQT

        def attn_stage(qt):
            qs = slice(qt * QT, (qt + 1) * QT)
            x_t = px.tile([128, HC, QT], BF16, tag="x")
            nc.sync.dma_start(x_t[:], x_in.ap()[:, :, qs])
            kt_t = pkt.tile([128, HC, W], BF16, tag="kt")
            nc.sync.dma_start(kt_t[:], kt_in.ap()[qt])
            cuw_t = pcuw.tile([128, n_kvc, H], BF16, tag="cuw")
            nc.sync.dma_start(cuw_t[:], cuw_in.ap()[qt])
            g_t = pg.tile([128, n_kvc, W], BF16, tag="g")
            nc.sync.dma_start(g_t[:], g_in.ap()[qt])
            m_t = pm.tile([128, n_kvc, QT], BF16, tag="m")
            nc.sync.dma_start(m_t[:], m_in.ap()[qt])

            # ---- rms_in stats: c = rsqrt(mean(x^2) + eps) per query
            ssq_ps = prow.tile([1, QT], F32, tag="row")
            for hc in range(HC):
                x2 = px2.tile([128, QT], BF16, tag="x2")
                nc.vector.tensor_tensor(x2, x_t[:, hc, :], x_t[:, hc, :], MUL)
                nc.tensor.matmul(ssq_ps, lhsT=ones_bf, rhs=x2,
                                 start=(hc == 0), stop=(hc == HC - 1))
            sd_row = prows.tile([1, QT], F32, tag="rows")
            nc.scalar.activation(sd_row, ssq_ps, AF.Sqrt, bias=eps_t,
                                 scale=1.0 / H)
            c_row = prows.tile([1, QT], BF16, tag="rowsb")
            with nc.allow_low_precision(reason="bf16 per-query scale factor"):
                nc.vector.reciprocal(c_row, sd_row)
            cb_ps = pbig.tile([128, QT], F32, tag="big")
            nc.tensor.matmul(cb_ps, lhsT=ones_row, rhs=c_row,
                             start=True, stop=True)
            c_b = pbc.tile([128, QT], F32, tag="bc")
            nc.vector.tensor_copy(c_b, cb_ps)

            # ---- scores -> pu = exp(s*c) * mask   (kv-major [W, QT])
            pm_t = ppm.tile([128, n_kvc, QT], BF16, tag="pm")
            for kvc in range(n_kvc):
                s_ps = pbig.tile([128, QT], F32, tag="big")
                for hc in range(HC):
                    nc.tensor.matmul(
                        s_ps, lhsT=kt_t[:, hc, kvc * 128:(kvc + 1) * 128],
                        rhs=x_t[:, hc, :],
                        start=(hc == 0), stop=(hc == HC - 1))
                t_sb = pt.tile([128, QT], F32, tag="t")
                nc.vector.tensor_tensor(t_sb, s_ps, c_b, MUL)
                pu = ppu.tile([128, QT], BF16, tag="pu")
                nc.scalar.activation(pu, t_sb, AF.Exp)
                nc.vector.tensor_tensor(pm_t[:, kvc, :], pu, m_t[:, kvc, :], MUL)

            # ---- z = sum_kv pu ; eps*z^2
            z_ps = prow.tile([1, QT], F32, tag="row")
            for kvc in range(n_kvc):
                nc.tensor.matmul(z_ps, lhsT=ones_bf, rhs=pm_t[:, kvc, :],
                                 start=(kvc == 0), stop=(kvc == n_kvc - 1))
            z2e = prows.tile([1, QT], F32, tag="rows")
            nc.scalar.activation(z2e, z_ps, AF.Square, scale=float(np.sqrt(EPS)))

            # ---- w' = pu G' pu^T via qhat = G' @ pu, pq = pu*qhat, reduce
            pq_t = ppq.tile([128, n_kvc, QT], BF16, tag="pq")
            for ko in range(n_kvc):
                q_ps = pbig.tile([128, QT], F32, tag="big")
                for ki in range(n_kvc):
                    nc.tensor.matmul(
                        q_ps, lhsT=g_t[:, ki, ko * 128:(ko + 1) * 128],
                        rhs=pm_t[:, ki, :],
                        start=(ki == 0), stop=(ki == n_kvc - 1))
                nc.vector.tensor_tensor(pq_t[:, ko, :], pm_t[:, ko, :], q_ps, MUL)
            w_ps = prow.tile([1, QT], F32, tag="row")
            for kvc in range(n_kvc):
                nc.tensor.matmul(w_ps, lhsT=ones_bf, rhs=pq_t[:, kvc, :],
                                 start=(kvc == 0), stop=(kvc == n_kvc - 1))

            # ---- alpha = rsqrt(w' + eps*z^2) ; ptil = pu * alpha
            v_row = prows.tile([1, QT], F32, tag="rows")
            nc.vector.tensor_tensor(v_row, w_ps, z2e, ADD)
            sq_row = prows.tile([1, QT], F32, tag="rows")
            nc.scalar.activation(sq_row, v_row, AF.Sqrt)
            al_row = prows.tile([1, QT], BF16, tag="rowsb")
            with nc.allow_low_precision(reason="bf16 per-query scale factor"):
                nc.vector.reciprocal(al_row, sq_row)
            ab_ps = pbig.tile([128, QT], F32, tag="big")
            nc.tensor.matmul(ab_ps, lhsT=ones_row, rhs=al_row,
                             start=True, stop=True)
            ptil_t = pptil.tile([128, n_kvc, QT], BF16, tag="ptil")
            for kvc in range(n_kvc):
                nc.vector.tensor_tensor(ptil_t[:, kvc, :], pm_t[:, kvc, :],
                                        ab_ps, MUL)
            st_x[qt] = x_t
            st_cuw[qt] = cuw_t
            st_ptil[qt] = ptil_t

        def out_stage(qt):
            qs = slice(qt * QT, (qt + 1) * QT)
            x_t, cuw_t, ptil_t = st_x[qt], st_cuw[qt], st_ptil[qt]
            for mc in range(MC):
                o_ps = pout.tile([128, QT], F32, tag="o")
                for hc in range(HC):
                    nc.tensor.matmul(o_ps,
                                     lhsT=wm2_sb[:, hc, mc * 128:(mc + 1) * 128],
                                     rhs=x_t[:, hc, :],
                                     start=(hc == 0), stop=False)
                for kvc in range(n_kvc):
                    nc.tensor.matmul(o_ps,
                                     lhsT=cuw_t[:, kvc, mc * 128:(mc + 1) * 128],
                                     rhs=ptil_t[:, kvc, :],
                                     start=False, stop=(kvc == n_kvc - 1))
                o_sb = po.tile([128, QT], BF16, tag="o")
                nc.scalar.activation(o_sb, o_ps, AF.Copy)
                nc.sync.dma_start(out_d.ap()[mc][:, qs], o_sb[:])

        # software pipeline: attention chain one tile ahead of the out stage
        attn_stage(0)
        for qt in range(1, NQT):
            attn_stage(qt)
            out_stage(qt - 1)
        out_stage(NQT - 1)

    nc.compile()
    return nc


def _get_program(W):
    if W not in _PROGRAM_CACHE:
        _PROGRAM_CACHE[W] = _build_program(W)
    return _PROGRAM_CACHE[W]


def kernel(**inputs) -> np.ndarray:
    global LAST_RESULTS
    inp = np.asarray(inputs["input"], np.float32)
    fw = np.asarray(inputs["fw"]).astype(np.int64)
    seq_sort = np.asarray(inputs["seq_sort"]).astype(np.int64)
    keep_cols = np.asarray(inputs["keep_cols"]).astype(np.int64)
    emb_alloc = np.asarray(inputs["emb_alloc"]).astype(np.int64)
    starts = np.asarray(inputs["starts"]).astype(np.int64)
    ends = np.asarray(inputs["ends"]).astype(np.int64)
    bb = int(np.asarray(inputs["bb"]))
    w_k = np.asarray(inputs["w_k_weight"], np.float32)
    w_v = np.asarray(inputs["w_v_weight"], np.float32)
    w_up = np.asarray(inputs["w_up_weight"], np.float32)
    w_mix = np.asarray(inputs["w_mix_weight"], np.float32)
    w_in = np.asarray(inputs["norm_in_weight"], np.float32)
    w_out = np.asarray(inputs["norm_out_weight"], np.float32)

    x = inp.reshape(BT, H)
    nb = BT // bb
    st = starts.reshape(nb, bb).min(axis=1)
    en = ends.reshape(nb, bb).max(axis=1)

    # sort block-rows j by label (stable); row s of sorted space = block-row
    # order[s] = query fw[order[s]]
    order = np.argsort(seq_sort, kind="stable")
    perm = fw[order]                         # original flat query per sorted row
    lab_q = seq_sort[order]                  # label per sorted row
    blk_q = order // bb
    st_q = st[blk_q]
    en_q = en[blk_q]
    x_sorted = x[perm]                       # [BT, H]

    # kv side: keep + label-sort; fold norm_in into K
    la = emb_alloc[keep_cols]                # [M]
    M = la.shape[0]
    kv_order = np.argsort(la, kind="stable")
    la_s = la[kv_order]
    kvpos = kv_order                         # kept-position of sorted kv row
    Bm = (w_k[keep_cols] * w_in[None, :])[kv_order]   # [M, H]
    Cm = w_v[keep_cols][kv_order]            # [M, D_EMB]

    counts = np.bincount(la_s, minlength=64)
    gstart = np.concatenate([[0], np.cumsum(counts)])  # [65]

    # per-tile windows over sorted kv
    NT = BT // QT                            # 32 global q-tiles
    win = np.empty(NT, np.int64)
    need = 0
    for g in range(NT):
        l0 = lab_q[g * QT]
        l1 = lab_q[(g + 1) * QT - 1]
        win[g] = gstart[l0]
        need = max(need, gstart[l1 + 1] - gstart[l0])
    W = max(256, int(-(-need // 128) * 128))

    # padded kv arrays so windows never go OOB
    Mp = M + W
    Bm_p = np.zeros((Mp, H), np.float32); Bm_p[:M] = Bm
    Cm_p = np.zeros((Mp, D_EMB), np.float32); Cm_p[:M] = Cm
    la_p = np.full(Mp, -1, np.int64); la_p[:M] = la_s
    kvpos_p = np.full(Mp, -1, np.int64); kvpos_p[:M] = kvpos

    # collapse comb->up->rms->mix_up through the label structure:
    CU = Cm_p @ w_up.T                                   # [Mp, D_UP]
    Wm1w = w_mix[:, :D_UP] * w_out[None, :]              # [H, D_UP]
    CUW = CU @ Wm1w.T                                    # [Mp, H]
    Wm2T = np.ascontiguousarray(w_mix[:, D_UP:].T)       # [H, H] (contr-major)

    # mask (1 valid / 0 invalid) per (sorted row, window col)
    kvi = win[:, None] + np.arange(W)[None, :]           # [NT, W]
    la_w = la_p[kvi]                                     # [NT, W]
    kp_w = kvpos_p[kvi]
    lab_t = lab_q.reshape(NT, QT)
    st_t = st_q.reshape(NT, QT)
    en_t = en_q.reshape(NT, QT)
    valid = ((la_w[:, None, :] == lab_t[:, :, None])
             & (kp_w[:, None, :] >= st_t[:, :, None])
             & (kp_w[:, None, :] < en_t[:, :, None]))    # [NT, QT, W]
    mask01 = valid.astype(np.float32)

    KT_full = np.ascontiguousarray(Bm_p.T)               # [H, Mp]

    wm2_host = np.ascontiguousarray(
        Wm2T.reshape(HC, 128, H).transpose(1, 0, 2)).astype(NP_BF16)  # [128,HC,H]

    n_kvc = W // 128
    in_maps = []
    for c in range(NC):
        rows = slice(c * NQ, (c + 1) * NQ)
        x_c = np.ascontiguousarray(
            x_sorted[rows].T.reshape(HC, 128, NQ).transpose(1, 0, 2)).astype(NP_BF16)
        kt_c = np.empty((NQT, 128, HC, W), NP_BF16)
        cuw_c = np.empty((NQT, 128, n_kvc, H), NP_BF16)
        g_c = np.empty((NQT, 128, n_kvc, W), NP_BF16)
        m_c = np.empty((NQT, 128, n_kvc, QT), NP_BF16)
        for qt in range(NQT):
            g = c * NQT + qt
            w0 = win[g]
            kt_c[qt] = KT_full[:, w0:w0 + W].reshape(HC, 128, W).transpose(1, 0, 2)
            cuw_c[qt] = CUW[w0:w0 + W].reshape(n_kvc, 128, H).transpose(1, 0, 2)
            Gwin = (CU[w0:w0 + W] @ CU[w0:w0 + W].T) * (1.0 / D_UP)   # [W, W]
            g_c[qt] = Gwin.reshape(n_kvc, 128, W).transpose(1, 0, 2)
            m_c[qt] = mask01[g].T.reshape(n_kvc, 128, QT).transpose(1, 0, 2)
        in_maps.append({
            "x_in": x_c, "kt_in": kt_c, "cuw_in": cuw_c, "g_in": g_c,
            "m_in": m_c, "wm2_in": wm2_host,
        })

    nc = _get_program(W)
    import time as _time
    global LAST_EXEC_S
    _t0 = _time.time()
    LAST_RESULTS = bass_utils.run_bass_kernel_spmd(nc, in_maps,
                                                   core_ids=list(range(NC)))
    LAST_EXEC_S = _time.time() - _t0
    out_sorted = np.concatenate(
        [np.asarray(r["out_d"], np.float32).transpose(2, 0, 1).reshape(NQ, H)
         for r in LAST_RESULTS.results],
        axis=0)                                          # [BT, H]
    final = np.empty((BT, H), np.float32)
    final[perm] = out_sorted
    return final.reshape(B, T, H)


# revision 12
# speedup vs baseline: 2.7544x; 1.0345x over previous
"""Trainium2 Bass kernel for nn_L3_31799937859925 (sparse_attention).

Strategy (v3 — low-rank collapse of the up/mix chain):
- Each query row (label = seq_sort[j] in [0,64)) attends only to kv rows with
  emb_alloc == label, so we sort queries by label on the host and give each of
  the 8 cores a contiguous 2048-query slice (pure data parallel, no
  collectives). kv rows are label-sorted too, so each 512-query tile only needs
  a small contiguous kv window (W columns) + a multiplicative {0,1} mask.
- Because comb = softmax(scores) @ C lives in the span of each label's 64 C
  rows, the entire comb -> up -> rms_out -> mix_up chain collapses through
  host-precomputed per-window operators:
      CUW = (C @ w_up.T) @ (w_mix[:, :d_up] * w_out).T      [kv, H]
      Ghat = (C @ w_up.T)(C @ w_up.T).T / d_up + eps        [kv, kv]
  With pu = masked exp(scores * inv_rms_in) and w'' = pu Ghat pu^T
  (= |up|^2/d_up + eps * z^2 since mask zeros kill cross-label terms):
      out = (pu * rsqrt(w'')) @ CUW + x @ w_mix[:, d_up:].T
  which removes ~13 of 18.8 GF/core from the device.
- Everything is feature-major ([feature, query]); per-query scalars broadcast
  across partitions via K=1 matmuls; partition reductions via ones-matmuls.
- All heavy matmuls run in bf16 (full PE rate, half DMA); accumulation f32.
- Software pipeline: the attention chain of tile t+1 is emitted before the
  output stage of tile t, so the PE never waits on the softmax scalar chain.
  All inputs are DMA'd upfront (everything fits in SBUF); one output DMA per
  tile.
"""
import numpy as np
import ml_dtypes

import concourse.bass as bass
import concourse.tile as tile
from concourse import bacc, mybir
import concourse.bass_utils as bass_utils

F32 = mybir.dt.float32
F32R = mybir.dt.float32r
BF16 = mybir.dt.bfloat16
AF = mybir.ActivationFunctionType
MUL = mybir.AluOpType.mult
ADD = mybir.AluOpType.add
NP_BF16 = ml_dtypes.bfloat16

H, N_EMB, D_EMB, D_UP = 1024, 8192, 512, 2048
B, T = 4, 4096
BT = B * T                  # 16384
NC = 8                      # cores
NQ = BT // NC               # 2048 queries per core
QT = 512                    # queries per q-tile
NQT = NQ // QT              # 4 q-tiles per core
HC = H // 128               # 8
MC = H // 128               # 8 output chunks
EPS = 1e-6

LAST_RESULTS = None         # BassKernelResults of the most recent run (for test.py)
LAST_EXEC_S = None
_PROGRAM_CACHE = {}


def _build_program(W):
    """Build the SPMD single-core program. W = kv window width (mult of 128)."""
    n_kvc = W // 128
    nc = bacc.Bacc("TRN2", target_bir_lowering=False, debug=False,
                   enable_asserts=False)

    x_in = nc.dram_tensor("x_in", [128, HC, NQ], BF16, kind="ExternalInput")
    kt_in = nc.dram_tensor("kt_in", [NQT, 128, HC, W], BF16, kind="ExternalInput")
    cuw_in = nc.dram_tensor("cuw_in", [NQT, 128, n_kvc, H], BF16, kind="ExternalInput")
    g_in = nc.dram_tensor("g_in", [NQT, 128, n_kvc, W], BF16, kind="ExternalInput")
    m_in = nc.dram_tensor("m_in", [NQT, 128, n_kvc, QT], BF16, kind="ExternalInput")
    wm2_in = nc.dram_tensor("wm2_in", [128, HC, H], BF16, kind="ExternalInput")
    out_d = nc.dram_tensor("out_d", [128, MC, NQ], BF16, kind="ExternalOutput")

    from contextlib import ExitStack
    with tile.TileContext(nc) as tc, ExitStack() as ctx:
        ec = ctx.enter_context
        cst = ec(tc.tile_pool(name="cst", bufs=1))
        pwm2 = ec(tc.tile_pool(name="wm2", bufs=1))
        px = ec(tc.tile_pool(name="px", bufs=NQT))
        pkt = ec(tc.tile_pool(name="pkt", bufs=NQT))
        pcuw = ec(tc.tile_pool(name="pcuw", bufs=NQT))
        pg = ec(tc.tile_pool(name="pg", bufs=NQT))
        pm = ec(tc.tile_pool(name="pm", bufs=NQT))
        px2 = ec(tc.tile_pool(name="px2", bufs=2))
        pt = ec(tc.tile_pool(name="pt", bufs=4))
        ppu = ec(tc.tile_pool(name="ppu", bufs=2))
        ppm = ec(tc.tile_pool(name="ppm", bufs=2))
        ppq = ec(tc.tile_pool(name="ppq", bufs=2))
        pptil = ec(tc.tile_pool(name="pptil", bufs=3))
        prows = ec(tc.tile_pool(name="prows", bufs=4))
        po = ec(tc.tile_pool(name="po", bufs=2))
        pbig = ec(tc.tile_pool(name="pbig", bufs=4, space="PSUM"))
        pout = ec(tc.tile_pool(name="pout", bufs=2, space="PSUM"))
        prow = ec(tc.tile_pool(name="prow", bufs=2, space="PSUM"))

        ones_f = cst.tile([128, 1], F32)
        nc.vector.memset(ones_f, 1.0)
        ones_bf = cst.tile([128, 1], BF16)
        nc.vector.tensor_copy(ones_bf, ones_f)
        ones_rf = cst.tile([1, 128], F32)
        nc.vector.memset(ones_rf, 1.0)
        ones_row = cst.tile([1, 128], BF16)
        nc.vector.tensor_copy(ones_row, ones_rf)
        eps_t = cst.tile([1, 1], F32)
        nc.vector.memset(eps_t, EPS)

        # ---- all input DMAs upfront (everything stays resident in SBUF)
        xs, kts, cuws, gs, ms = [], [], [], [], []
        wm2_sb = None
        for qt in range(NQT):
            qs = slice(qt * QT, (qt + 1) * QT)
            x_t = px.tile([128, HC, QT], BF16, tag="x")
            nc.sync.dma_start(x_t[:], x_in.ap()[:, :, qs])
            kt_t = pkt.tile([128, HC, W], BF16, tag="kt")
            nc.sync.dma_start(kt_t[:], kt_in.ap()[qt])
            cuw_t = pcuw.tile([128, n_kvc, H], BF16, tag="cuw")
            nc.sync.dma_start(cuw_t[:], cuw_in.ap()[qt])
            g_t = pg.tile([128, n_kvc, W], BF16, tag="g")
            nc.sync.dma_start(g_t[:], g_in.ap()[qt])
            m_t = pm.tile([128, n_kvc, QT], BF16, tag="m")
            nc.sync.dma_start(m_t[:], m_in.ap()[qt])
            xs.append(x_t); kts.append(kt_t); cuws.append(cuw_t)
            gs.append(g_t); ms.append(m_t)
            if qt == 1:
                wm2_sb = pwm2.tile([128, HC, H], BF16)
                nc.sync.dma_start(wm2_sb[:], wm2_in.ap())

        st_ptil = [None] * NQT

        def attn_stage(qt):
            x_t, kt_t, g_t, m_t = xs[qt], kts[qt], gs[qt], ms[qt]

            # ---- rms_in stats: c = rsqrt(mean(x^2) + eps) per query
            ssq_ps = prow.tile([1, QT], F32, tag="row")
            for hc in range(HC):
                x2 = px2.tile([128, QT], BF16, tag="x2")
                nc.vector.tensor_tensor(x2, x_t[:, hc, :], x_t[:, hc, :], MUL)
                nc.tensor.matmul(ssq_ps, lhsT=ones_bf, rhs=x2,
                                 start=(hc == 0), stop=(hc == HC - 1))
            sd_row = prows.tile([1, QT], F32, tag="rows")
            nc.scalar.activation(sd_row, ssq_ps, AF.Sqrt, bias=eps_t,
                                 scale=1.0 / H)
            c_row = prows.tile([1, QT], BF16, tag="rowsb")
            with nc.allow_low_precision(reason="bf16 per-query scale factor"):
                nc.vector.reciprocal(c_row, sd_row)
            cb_ps = pbig.tile([128, QT], F32, tag="big")
            nc.tensor.matmul(cb_ps, lhsT=ones_row, rhs=c_row,
                             start=True, stop=True)
            c_b = pt.tile([128, QT], F32, tag="cb")
            nc.vector.tensor_copy(c_b, cb_ps)

            # ---- scores -> pu = exp(s*c) * mask   (kv-major [W, QT])
            pm_t = ppm.tile([128, n_kvc, QT], BF16, tag="pm")
            for kvc in range(n_kvc):
                s_ps = pbig.tile([128, QT], F32, tag="big")
                for hc in range(HC):
                    nc.tensor.matmul(
                        s_ps, lhsT=kt_t[:, hc, kvc * 128:(kvc + 1) * 128],
                        rhs=x_t[:, hc, :],
                        start=(hc == 0), stop=(hc == HC - 1))
                t_sb = pt.tile([128, QT], F32, tag="t")
                nc.vector.tensor_tensor(t_sb, s_ps, c_b, MUL)
                pu = ppu.tile([128, QT], BF16, tag="pu")
                nc.scalar.activation(pu, t_sb, AF.Exp)
                nc.vector.tensor_tensor(pm_t[:, kvc, :], pu, m_t[:, kvc, :], MUL)

            # ---- w'' = pu Ghat pu^T  (= |up|^2/d_up + eps*z^2)
            pq_t = ppq.tile([128, n_kvc, QT], BF16, tag="pq")
            for ko in range(n_kvc):
                q_ps = pbig.tile([128, QT], F32, tag="big")
                for ki in range(n_kvc):
                    nc.tensor.matmul(
                        q_ps, lhsT=g_t[:, ki, ko * 128:(ko + 1) * 128],
                        rhs=pm_t[:, ki, :],
                        start=(ki == 0), stop=(ki == n_kvc - 1))
                nc.vector.tensor_tensor(pq_t[:, ko, :], pm_t[:, ko, :], q_ps, MUL)
            w_ps = prow.tile([1, QT], F32, tag="row")
            for kvc in range(n_kvc):
                nc.tensor.matmul(w_ps, lhsT=ones_bf, rhs=pq_t[:, kvc, :],
                                 start=(kvc == 0), stop=(kvc == n_kvc - 1))

            # ---- alpha = rsqrt(w'') ; ptil = pu * alpha
            sq_row = prows.tile([1, QT], F32, tag="rows")
            nc.scalar.activation(sq_row, w_ps, AF.Sqrt)
            al_row = prows.tile([1, QT], BF16, tag="rowsb")
            with nc.allow_low_precision(reason="bf16 per-query scale factor"):
                nc.vector.reciprocal(al_row, sq_row)
            ab_ps = pbig.tile([128, QT], F32, tag="big")
            nc.tensor.matmul(ab_ps, lhsT=ones_row, rhs=al_row,
                             start=True, stop=True)
            ptil_t = pptil.tile([128, n_kvc, QT], BF16, tag="ptil")
            for kvc in range(n_kvc):
                nc.vector.tensor_tensor(ptil_t[:, kvc, :], pm_t[:, kvc, :],
                                        ab_ps, MUL)
            st_ptil[qt] = ptil_t

        def out_stage(qt):
            qs = slice(qt * QT, (qt + 1) * QT)
            x_t, cuw_t, ptil_t = xs[qt], cuws[qt], st_ptil[qt]
            o_sb = po.tile([128, MC, QT], BF16, tag="o")
            for mc in range(MC):
                o_ps = pout.tile([128, QT], F32, tag="o")
                for hc in range(HC):
                    nc.tensor.matmul(o_ps,
                                     lhsT=wm2_sb[:, hc, mc * 128:(mc + 1) * 128],
                                     rhs=x_t[:, hc, :],
                                     start=(hc == 0), stop=False)
                for kvc in range(n_kvc):
                    nc.tensor.matmul(o_ps,
                                     lhsT=cuw_t[:, kvc, mc * 128:(mc + 1) * 128],
                                     rhs=ptil_t[:, kvc, :],
                                     start=False, stop=(kvc == n_kvc - 1))
                nc.scalar.activation(o_sb[:, mc, :], o_ps, AF.Copy)
            nc.sync.dma_start(out_d.ap()[:, :, qs], o_sb[:])

        # software pipeline: attention chain one tile ahead of the out stage
        attn_stage(0)
        for qt in range(1, NQT):
            attn_stage(qt)
            out_stage(qt - 1)
        out_stage(NQT - 1)

    nc.compile()
    return nc


def _get_program(W):
    if W not in _PROGRAM_CACHE:
        _PROGRAM_CACHE[W] = _build_program(W)
    return _PROGRAM_CACHE[W]


def kernel(**inputs) -> np.ndarray:
    global LAST_RESULTS
    inp = np.asarray(inputs["input"], np.float32)
    fw = np.asarray(inputs["fw"]).astype(np.int64)
    seq_sort = np.asarray(inputs["seq_sort"]).astype(np.int64)
    keep_cols = np.asarray(inputs["keep_cols"]).astype(np.int64)
    emb_alloc = np.asarray(inputs["emb_alloc"]).astype(np.int64)
    starts = np.asarray(inputs["starts"]).astype(np.int64)
    ends = np.asarray(inputs["ends"]).astype(np.int64)
    bb = int(np.asarray(inputs["bb"]))
    w_k = np.asarray(inputs["w_k_weight"], np.float32)
    w_v = np.asarray(inputs["w_v_weight"], np.float32)
    w_up = np.asarray(inputs["w_up_weight"], np.float32)
    w_mix = np.asarray(inputs["w_mix_weight"], np.float32)
    w_in = np.asarray(inputs["norm_in_weight"], np.float32)
    w_out = np.asarray(inputs["norm_out_weight"], np.float32)

    x = inp.reshape(BT, H)
    nb = BT // bb
    st = starts.reshape(nb, bb).min(axis=1)
    en = ends.reshape(nb, bb).max(axis=1)

    # sort block-rows j by label (stable); row s of sorted space = block-row
    # order[s] = query fw[order[s]]
    order = np.argsort(seq_sort, kind="stable")
    perm = fw[order]                         # original flat query per sorted row
    lab_q = seq_sort[order]                  # label per sorted row
    blk_q = order // bb
    st_q = st[blk_q]
    en_q = en[blk_q]
    x_sorted = x[perm]                       # [BT, H]

    # kv side: keep + label-sort; fold norm_in into K
    la = emb_alloc[keep_cols]                # [M]
    M = la.shape[0]
    kv_order = np.argsort(la, kind="stable")
    la_s = la[kv_order]
    kvpos = kv_order                         # kept-position of sorted kv row
    Bm = (w_k[keep_cols] * w_in[None, :])[kv_order]   # [M, H]
    Cm = w_v[keep_cols][kv_order]            # [M, D_EMB]

    counts = np.bincount(la_s, minlength=64)
    gstart = np.concatenate([[0], np.cumsum(counts)])  # [65]

    # per-tile windows over sorted kv
    NT = BT // QT                            # 32 global q-tiles
    win = np.empty(NT, np.int64)
    need = 0
    for g in range(NT):
        l0 = lab_q[g * QT]
        l1 = lab_q[(g + 1) * QT - 1]
        win[g] = gstart[l0]
        need = max(need, gstart[l1 + 1] - gstart[l0])
    W = max(256, int(-(-need // 128) * 128))

    # padded kv arrays so windows never go OOB
    Mp = M + W
    Bm_p = np.zeros((Mp, H), np.float32); Bm_p[:M] = Bm
    Cm_p = np.zeros((Mp, D_EMB), np.float32); Cm_p[:M] = Cm
    la_p = np.full(Mp, -1, np.int64); la_p[:M] = la_s
    kvpos_p = np.full(Mp, -1, np.int64); kvpos_p[:M] = kvpos

    # collapse comb->up->rms->mix_up through the label structure:
    CU = Cm_p @ w_up.T                                   # [Mp, D_UP]
    Wm1w = w_mix[:, :D_UP] * w_out[None, :]              # [H, D_UP]
    CUW = CU @ Wm1w.T                                    # [Mp, H]
    Wm2T = np.ascontiguousarray(w_mix[:, D_UP:].T)       # [H, H] (contr-major)

    # mask (1 valid / 0 invalid) per (sorted row, window col)
    kvi = win[:, None] + np.arange(W)[None, :]           # [NT, W]
    la_w = la_p[kvi]                                     # [NT, W]
    kp_w = kvpos_p[kvi]
    lab_t = lab_q.reshape(NT, QT)
    st_t = st_q.reshape(NT, QT)
    en_t = en_q.reshape(NT, QT)
    valid = ((la_w[:, None, :] == lab_t[:, :, None])
             & (kp_w[:, None, :] >= st_t[:, :, None])
             & (kp_w[:, None, :] < en_t[:, :, None]))    # [NT, QT, W]
    mask01 = valid.astype(np.float32)

    KT_full = np.ascontiguousarray(Bm_p.T)               # [H, Mp]

    wm2_host = np.ascontiguousarray(
        Wm2T.reshape(HC, 128, H).transpose(1, 0, 2)).astype(NP_BF16)  # [128,HC,H]

    n_kvc = W // 128
    in_maps = []
    for c in range(NC):
        rows = slice(c * NQ, (c + 1) * NQ)
        x_c = np.ascontiguousarray(
            x_sorted[rows].T.reshape(HC, 128, NQ).transpose(1, 0, 2)).astype(NP_BF16)
        kt_c = np.empty((NQT, 128, HC, W), NP_BF16)
        cuw_c = np.empty((NQT, 128, n_kvc, H), NP_BF16)
        g_c = np.empty((NQT, 128, n_kvc, W), NP_BF16)
        m_c = np.empty((NQT, 128, n_kvc, QT), NP_BF16)
        for qt in range(NQT):
            g = c * NQT + qt
            w0 = win[g]
            kt_c[qt] = KT_full[:, w0:w0 + W].reshape(HC, 128, W).transpose(1, 0, 2)
            cuw_c[qt] = CUW[w0:w0 + W].reshape(n_kvc, 128, H).transpose(1, 0, 2)
            Gwin = (CU[w0:w0 + W] @ CU[w0:w0 + W].T) * (1.0 / D_UP) + EPS
            g_c[qt] = Gwin.reshape(n_kvc, 128, W).transpose(1, 0, 2)
            m_c[qt] = mask01[g].T.reshape(n_kvc, 128, QT).transpose(1, 0, 2)
        in_maps.append({
            "x_in": x_c, "kt_in": kt_c, "cuw_in": cuw_c, "g_in": g_c,
            "m_in": m_c, "wm2_in": wm2_host,
        })

    nc = _get_program(W)
    import time as _time
    global LAST_EXEC_S
    _t0 = _time.time()
    LAST_RESULTS = bass_utils.run_bass_kernel_spmd(nc, in_maps,
                                                   core_ids=list(range(NC)))
    LAST_EXEC_S = _time.time() - _t0
    out_sorted = np.concatenate(
        [np.asarray(r["out_d"], np.float32).transpose(1, 0, 2)
         .reshape(H, NQ).T.reshape(NQ, H)
         for r in LAST_RESULTS.results],
        axis=0)                                          # [BT, H]
    final = np.empty((BT, H), np.float32)
    final[perm] = out_sorted
    return final.reshape(B, T, H)
